# revision 16
# baseline (speedup 1.0000x reference)
"""Trainium2 Bass kernel for nn_Attention_62130996904205.

Full computation (reference):
    q = left @ Wq;  k,v = split(right @ Wkv)
    per head: S = scale * q k^T; S = where(mask, S, -1e7)
    out = (softmax(S) @ v) rearranged @ Wout + bout

Sharding: 8 cores = (batch b in 0..3) x (head-half in 0..1).
Each core computes, for its batch and its 4 heads, the partial
out = concat_h(softmax_h @ v_h) @ Wout[head-rows]; host sums the two
head-half partials per batch and adds bout.

On-chip layout ("S^T scheme"): all attention matmuls keep the kv token
index n on the partition axis:
    q^T (DH, M), k^T (DH, N) from projections of pre-transposed
    left^T/right^T;  S^T tile = k_tile @ q^T  (PSUM, (128, M-chunk));
    P^T = exp(S^T) * mask^T  (bf16, SBUF);
    O^T = sum_nt [v_nt | 1]^T @ P^T_nt  (PSUM (65, M-chunk); row 64
    accumulates the softmax denominator);
    U^T = O^T[0:64] / d broadcast;  out = (U stacked over heads) @ Wout.

Host pre-transposes left/right/mask (layout choice for sharding) and
converts to bf16; the 1/sqrt(DH) scale is folded into Wq.
"""

import numpy as np
import ml_dtypes

import concourse.bass as bass
import concourse.mybir as mybir
import concourse.tile as tile
from concourse import bacc
from concourse.bass_utils import run_bass_kernel_spmd

BF16 = ml_dtypes.bfloat16
FP32 = np.float32

# set by test harness to enable NTFF tracing
TRACE = False
LAST_RESULTS = None


def build_core(M=1024, N=4096, DQ=512, H=4, DH=64):
    """Build the per-core Bass program. Every core runs this same program
    on its own shard (SPMD)."""
    dt = mybir.dt
    f32, bf16 = dt.float32, dt.bfloat16
    D = H * DH            # head features handled by this core
    KT = DQ // 128        # contraction tiles for the projections
    NT = N // 128         # kv-token tiles
    NTP = NT // 2         # processed in pairs of two 128-tiles
    MCH = min(512, M)     # m-chunk (PSUM free width per O accumulator)
    NMC = M // MCH
    SW = 2 * MCH          # S-psum width: one nt-pair worth of logits
    DA = DH + 1           # v augmented with a ones column (denominator)
    KT2 = D // 128        # contraction tiles for the output projection
    VW = H * DH           # v-projection free width
    VPACK = min(NT, max(1, SW // VW))  # nt's packed per v-projection psum tile

    assert M % MCH == 0 and N % 256 == 0 and DQ % 128 == 0 and D % 128 == 0

    nc = bacc.Bacc("TRN2", target_bir_lowering=False, debug=False)

    leftT = nc.dram_tensor("leftT", [DQ, M], bf16, kind="ExternalInput")
    rightT = nc.dram_tensor("rightT", [DQ, N], bf16, kind="ExternalInput")
    maskT = nc.dram_tensor("maskT", [N, M], bf16, kind="ExternalInput")
    wq = nc.dram_tensor("wq", [DQ, D], bf16, kind="ExternalInput")
    wk = nc.dram_tensor("wk", [DQ, D], bf16, kind="ExternalInput")
    wv = nc.dram_tensor("wv", [DQ, D], bf16, kind="ExternalInput")
    wout = nc.dram_tensor("wout", [D, DQ], bf16, kind="ExternalInput")
    out_p = nc.dram_tensor("out_p", [M, DQ], f32, kind="ExternalOutput")

    EXP = mybir.ActivationFunctionType.Exp

    with tile.TileContext(nc) as tc:
        with (
            tc.tile_pool(name="sing", bufs=1) as sing,
            tc.tile_pool(name="spool", bufs=3, space="PSUM") as spool,
            tc.tile_pool(name="opool", bufs=1, space="PSUM") as opool,
            tc.tile_pool(name="mpool", bufs=min(NTP, 16) + 2) as mpool,
            tc.tile_pool(name="ppool", bufs=8) as ppool,
            tc.tile_pool(name="smallp", bufs=2) as smallp,
            tc.tile_pool(name="outp", bufs=3) as outp,
        ):
            # ---- weight + activation loads -------------------------------
            wq_sb = sing.tile([128, KT, D], bf16, tag="wq")
            nc.sync.dma_start(out=wq_sb, in_=wq.rearrange("(kt p) d -> p kt d", p=128))
            wk_sb = sing.tile([128, KT, D], bf16, tag="wk")
            nc.sync.dma_start(out=wk_sb, in_=wk.rearrange("(kt p) d -> p kt d", p=128))
            wv_sb = sing.tile([128, KT, D], bf16, tag="wv")
            nc.sync.dma_start(out=wv_sb, in_=wv.rearrange("(kt p) d -> p kt d", p=128))
            wout_sb = sing.tile([128, KT2, DQ], bf16, tag="wout")
            nc.sync.dma_start(
                out=wout_sb, in_=wout.rearrange("(kt p) d -> p kt d", p=128)
            )

            leftT_sb = []
            for kt in range(KT):
                t = sing.tile([128, M], bf16, tag=f"leftT{kt}", name=f"leftT{kt}")
                nc.sync.dma_start(out=t, in_=leftT[kt * 128 : (kt + 1) * 128, :])
                leftT_sb.append(t)
            rightT_sb = []
            for kt in range(KT):
                t = sing.tile([128, N], bf16, tag=f"rightT{kt}", name=f"rightT{kt}")
                nc.sync.dma_start(out=t, in_=rightT[kt * 128 : (kt + 1) * 128, :])
                rightT_sb.append(t)

            # ---- projections ---------------------------------------------
            # q^T (per head: (64, M)) and k^T (per head: (64, N)); heads are
            # produced two at a time (128 psum partitions).
            qT = [sing.tile([64, M], bf16, tag=f"qT{h}", name=f"qT{h}") for h in range(H)]
            kT = [sing.tile([64, N], bf16, tag=f"kT{h}", name=f"kT{h}") for h in range(H)]
            u_sb = [sing.tile([128, M], bf16, tag=f"u{p}", name=f"u{p}") for p in range(KT2)]
            v_aug = sing.tile([128, NT, H, DA], bf16, tag="vaug")
            nc.vector.memset(v_aug[:, :, :, DH : DH + 1], 1.0)

            # q projection
            for t2 in range(H // 2):
                ps = spool.tile([128, SW], f32, tag="s")
                w512 = min(512, M)
                for mh in range(M // w512):
                    for kt in range(KT):
                        nc.tensor.matmul(
                            ps[:, mh * w512 : (mh + 1) * w512],
                            lhsT=wq_sb[:, kt, t2 * 128 : (t2 + 1) * 128],
                            rhs=leftT_sb[kt][:, mh * w512 : (mh + 1) * w512],
                            start=(kt == 0),
                            stop=(kt == KT - 1),
                        )
                nc.scalar.copy(out=qT[2 * t2][:, :], in_=ps[0:64, 0:M])
                nc.scalar.copy(out=qT[2 * t2 + 1][:, :], in_=ps[64:128, 0:M])

            # k projection
            CW = min(SW, N)
            for t2 in range(H // 2):
                for cp in range(N // CW):
                    ps = spool.tile([128, SW], f32, tag="s")
                    w512 = min(512, CW)
                    for half in range(CW // w512):
                        for kt in range(KT):
                            nc.tensor.matmul(
                                ps[:, half * w512 : (half + 1) * w512],
                                lhsT=wk_sb[:, kt, t2 * 128 : (t2 + 1) * 128],
                                rhs=rightT_sb[kt][
                                    :, cp * CW + half * w512 : cp * CW + (half + 1) * w512
                                ],
                                start=(kt == 0),
                                stop=(kt == KT - 1),
                            )
                    nc.vector.tensor_copy(
                        out=kT[2 * t2][:, cp * CW : (cp + 1) * CW], in_=ps[0:64, 0:CW]
                    )
                    nc.vector.tensor_copy(
                        out=kT[2 * t2 + 1][:, cp * CW : (cp + 1) * CW],
                        in_=ps[64:128, 0:CW],
                    )

            # v projection (VPACK nt-tiles share one psum tile)
            for g in range(NT // VPACK):
                ps = spool.tile([128, SW], f32, tag="s")
                for j in range(VPACK):
                    nt = g * VPACK + j
                    for kt in range(KT):
                        nc.tensor.matmul(
                            ps[:, j * VW : (j + 1) * VW],
                            lhsT=rightT_sb[kt][:, nt * 128 : (nt + 1) * 128],
                            rhs=wv_sb[:, kt, :],
                            start=(kt == 0),
                            stop=(kt == KT - 1),
                        )
                nc.vector.tensor_copy(
                    out=v_aug[:, g * VPACK : (g + 1) * VPACK, :, 0:DH],
                    in_=ps[:, 0 : VPACK * VW],
                )

            # ---- attention ----------------------------------------------
            # head-pair outer loop keeps PSUM pressure low (2 O accumulators
            # + 3 double-wide S tiles = 8 banks) so the S pipeline stays deep.
            for mc in range(NMC):
                msks = []
                for ntp in range(NTP):
                    msk = mpool.tile([128, SW], bf16, tag="msk", name=f"msk{ntp}")
                    nc.sync.dma_start(
                        out=msk,
                        in_=maskT[
                            ntp * 256 : (ntp + 1) * 256, mc * MCH : (mc + 1) * MCH
                        ].rearrange("(a p) f -> p a f", p=128),
                    )
                    msks.append(msk)
                for hp in range(H // 2):
                    o_ps = [
                        opool.tile([DA, MCH], f32, tag=f"o{i}", name=f"o{i}")
                        for i in range(2)
                    ]
                    # software pipeline: O-matmuls trail the S/exp/mask chain
                    # by DEPTH steps so the PE never stalls on exp/mask deps.
                    DEPTH = 4
                    oq = []          # deferred (pm, i, ntp)
                    started = [False, False]

                    def flush_one():
                        pm, i, ntp_ = oq.pop(0)
                        for half in range(2):
                            nt = 2 * ntp_ + half
                            nc.tensor.matmul(
                                o_ps[i],
                                lhsT=v_aug[:, nt, 2 * hp + i, :],
                                rhs=pm[:, half * MCH : (half + 1) * MCH],
                                start=(not started[i]),
                                stop=(ntp_ == NTP - 1 and half == 1),
                            )
                            started[i] = True

                    for ntp in range(NTP):
                        for i in range(2):
                            h = 2 * hp + i
                            s_ps = spool.tile([128, SW], f32, tag="s")
                            for half in range(2):
                                nt = 2 * ntp + half
                                nc.tensor.matmul(
                                    s_ps[:, half * MCH : (half + 1) * MCH],
                                    lhsT=kT[h][:, nt * 128 : (nt + 1) * 128],
                                    rhs=qT[h][:, mc * MCH : (mc + 1) * MCH],
                                    start=True,
                                    stop=True,
                                )
                            p_sb = ppool.tile([128, SW], bf16, tag="p")
                            nc.scalar.activation(p_sb, s_ps, EXP)
                            pm = ppool.tile([128, SW], bf16, tag="pm")
                            nc.vector.tensor_mul(pm, p_sb, msks[ntp])
                            oq.append((pm, i, ntp))
                            if len(oq) > DEPTH:
                                flush_one()
                    while oq:
                        flush_one()
                    # normalize: U^T = O^T[0:DH] * (1/d) broadcast over parts
                    for i in range(2):
                        h = 2 * hp + i
                        rdc = smallp.tile([1, MCH], f32, tag="rdc", name=f"rdc{i}")
                        nc.vector.tensor_copy(out=rdc, in_=o_ps[i][DH : DH + 1, :])
                        rd = smallp.tile([1, MCH], f32, tag="rd", name=f"rd{i}")
                        nc.vector.reciprocal_approx_fast(out=rd, in_=rdc)
                        bd = smallp.tile([64, MCH], f32, tag="bd", name=f"bd{i}")
                        nc.gpsimd.partition_broadcast(bd, rd)
                        nc.vector.tensor_mul(
                            u_sb[h // 2][
                                (h % 2) * 64 : (h % 2) * 64 + 64,
                                mc * MCH : (mc + 1) * MCH,
                            ],
                            o_ps[i][0:DH, :],
                            bd,
                        )

            # ---- output projection --------------------------------------
            for mt in range(M // 128):
                ps = spool.tile([128, SW], f32, tag="s")
                for p2 in range(KT2):
                    nc.tensor.matmul(
                        ps[:, 0:DQ],
                        lhsT=u_sb[p2][:, mt * 128 : (mt + 1) * 128],
                        rhs=wout_sb[:, p2, :],
                        start=(p2 == 0),
                        stop=(p2 == KT2 - 1),
                    )
                ob = outp.tile([128, DQ], f32, tag="ob")
                nc.scalar.copy(ob, ps[:, 0:DQ])
                nc.sync.dma_start(out=out_p[mt * 128 : (mt + 1) * 128, :], in_=ob)

    nc.finalize()
    return nc


_NC_CACHE = {}


def _get_nc(key=(1024, 4096, 512, 4, 64)):
    if key not in _NC_CACHE:
        _NC_CACHE[key] = build_core(*key)
    return _NC_CACHE[key]


def kernel(left, right, mask, Wq, Wkv, Wout, bout):
    """Full-input entry point: shards across 8 neuron cores, returns the
    full (B, M, DQ) output."""
    global LAST_RESULTS
    B, M, DQmat = left.shape
    _, N, DC = right.shape
    H, DH = 8, 64
    D = H * DH
    Hc = H // 2          # heads per core
    scale = DH ** -0.5

    left = np.asarray(left, dtype=np.float32)
    right = np.asarray(right, dtype=np.float32)
    Wq = np.asarray(Wq, dtype=np.float32)
    Wkv = np.asarray(Wkv, dtype=np.float32)
    Wout = np.asarray(Wout, dtype=np.float32)
    bout = np.asarray(bout, dtype=np.float32)

    Wqs = (Wq * scale).astype(BF16)            # (DQ, D)
    Wk = Wkv[:, :D].astype(BF16)               # (DC, D)
    Wv = Wkv[:, D:].astype(BF16)               # (DC, D)
    WoutB = Wout.astype(BF16)                  # (D, DQ)

    leftT = np.ascontiguousarray(left.transpose(0, 2, 1)).astype(BF16)    # (B, DQ, M)
    rightT = np.ascontiguousarray(right.transpose(0, 2, 1)).astype(BF16)  # (B, DC, N)
    maskT = np.ascontiguousarray(mask.transpose(0, 2, 1)).astype(BF16)    # (B, N, M)

    nc = _get_nc((M, N, DQmat, Hc, DH))

    in_maps = []
    for core in range(8):
        b, hh = divmod(core, 2)
        hs = slice(hh * Hc * DH, (hh + 1) * Hc * DH)
        in_maps.append(
            {
                "leftT": leftT[b],
                "rightT": rightT[b],
                "maskT": maskT[b],
                "wq": np.ascontiguousarray(Wqs[:, hs]),
                "wk": np.ascontiguousarray(Wk[:, hs]),
                "wv": np.ascontiguousarray(Wv[:, hs]),
                "wout": np.ascontiguousarray(WoutB[hs, :]),
            }
        )

    tmpdir = None
    if TRACE:
        import shutil, tempfile

        shutil.rmtree("/tmp/attn_trace", ignore_errors=True)
        tmpdir = "/tmp/attn_trace"
    res = run_bass_kernel_spmd(nc, in_maps, list(range(8)), trace=TRACE, tmpdir=tmpdir)
    LAST_RESULTS = res

    out = np.zeros((B, M, DQmat), np.float32)
    for core in range(8):
        out[core // 2] += res.results[core]["out_p"]
    out += bout[None, None, :]
    return out


# revision 17
# speedup vs baseline: 1.0814x; 1.0814x over previous
"""Trainium2 Bass kernel for nn_Attention_62130996904205.

Full computation (reference):
    q = left @ Wq;  k,v = split(right @ Wkv)
    per head: S = scale * q k^T; S = where(mask, S, -1e7)
    out = (softmax(S) @ v) rearranged @ Wout + bout

Sharding: 8 cores = (batch b in 0..3) x (head-half in 0..1).
Each core computes, for its batch and its 4 heads, the partial
out = concat_h(softmax_h @ v_h) @ Wout[head-rows]; host sums the two
head-half partials per batch and adds bout.

On-chip layout ("S^T scheme"): all attention matmuls keep the kv token
index n on the partition axis:
    q^T (DH, M), k^T (DH, N) from projections of pre-transposed
    left^T/right^T;  S^T tile = k_tile @ q^T  (PSUM, (128, M-chunk));
    P^T = exp(S^T) * mask^T  (bf16, SBUF);
    O^T = sum_nt [v_nt | 1]^T @ P^T_nt  (PSUM (65, M-chunk); row 64
    accumulates the softmax denominator);
    U^T = O^T[0:64] / d broadcast;  out = (U stacked over heads) @ Wout.

Host pre-transposes left/right/mask (layout choice for sharding) and
converts to bf16; the 1/sqrt(DH) scale is folded into Wq.
"""

import numpy as np
import ml_dtypes

import concourse.bass as bass
import concourse.mybir as mybir
import concourse.tile as tile
from concourse import bacc
from concourse.bass_utils import run_bass_kernel_spmd

BF16 = ml_dtypes.bfloat16
FP32 = np.float32

# set by test harness to enable NTFF tracing
TRACE = False
LAST_RESULTS = None


def build_core(M=1024, N=4096, DQ=512, H=4, DH=64):
    """Build the per-core Bass program. Every core runs this same program
    on its own shard (SPMD)."""
    dt = mybir.dt
    f32, bf16 = dt.float32, dt.bfloat16
    D = H * DH            # head features handled by this core
    KT = DQ // 128        # contraction tiles for the projections
    NT = N // 128         # kv-token tiles
    NTP = NT // 2         # processed in pairs of two 128-tiles
    MCH = min(512, M)     # m-chunk (PSUM free width per O accumulator)
    NMC = M // MCH
    SW = 2 * MCH          # S-psum width: one nt-pair worth of logits
    DA = DH + 1           # v augmented with a ones column (denominator)
    KT2 = D // 128        # contraction tiles for the output projection
    VW = H * DH           # v-projection free width
    VPACK = min(NT, max(1, SW // VW))  # nt's packed per v-projection psum tile

    assert M % MCH == 0 and N % 256 == 0 and DQ % 128 == 0 and D % 128 == 0

    nc = bacc.Bacc("TRN2", target_bir_lowering=False, debug=False)

    leftT = nc.dram_tensor("leftT", [DQ, M], bf16, kind="ExternalInput")
    rightT = nc.dram_tensor("rightT", [DQ, N], bf16, kind="ExternalInput")
    maskT = nc.dram_tensor("maskT", [N, M], bf16, kind="ExternalInput")
    wq = nc.dram_tensor("wq", [DQ, D], bf16, kind="ExternalInput")
    wk = nc.dram_tensor("wk", [DQ, D], bf16, kind="ExternalInput")
    wv = nc.dram_tensor("wv", [DQ, D], bf16, kind="ExternalInput")
    wout = nc.dram_tensor("wout", [D, DQ], bf16, kind="ExternalInput")
    out_p = nc.dram_tensor("out_p", [M, DQ], f32, kind="ExternalOutput")

    EXP = mybir.ActivationFunctionType.Exp

    with tile.TileContext(nc) as tc:
        with (
            tc.tile_pool(name="sing", bufs=1) as sing,
            tc.tile_pool(name="spool", bufs=3, space="PSUM") as spool,
            tc.tile_pool(name="opool", bufs=1, space="PSUM") as opool,
            tc.tile_pool(name="mpool", bufs=min(NTP, 16) + 2) as mpool,
            tc.tile_pool(name="ppool", bufs=8) as ppool,
            tc.tile_pool(name="smallp", bufs=2) as smallp,
            tc.tile_pool(name="outp", bufs=3) as outp,
        ):
            # ---- weight + activation loads -------------------------------
            wq_sb = sing.tile([128, KT, D], bf16, tag="wq")
            nc.sync.dma_start(out=wq_sb, in_=wq.rearrange("(kt p) d -> p kt d", p=128))
            wk_sb = sing.tile([128, KT, D], bf16, tag="wk")
            nc.sync.dma_start(out=wk_sb, in_=wk.rearrange("(kt p) d -> p kt d", p=128))
            wv_sb = sing.tile([128, KT, D], bf16, tag="wv")
            nc.sync.dma_start(out=wv_sb, in_=wv.rearrange("(kt p) d -> p kt d", p=128))
            wout_sb = sing.tile([128, KT2, DQ], bf16, tag="wout")
            nc.sync.dma_start(
                out=wout_sb, in_=wout.rearrange("(kt p) d -> p kt d", p=128)
            )

            leftT_sb = []
            for kt in range(KT):
                t = sing.tile([128, M], bf16, tag=f"leftT{kt}", name=f"leftT{kt}")
                nc.sync.dma_start(out=t, in_=leftT[kt * 128 : (kt + 1) * 128, :])
                leftT_sb.append(t)
            rightT_sb = []
            for kt in range(KT):
                t = sing.tile([128, N], bf16, tag=f"rightT{kt}", name=f"rightT{kt}")
                nc.scalar.dma_start(out=t, in_=rightT[kt * 128 : (kt + 1) * 128, :])
                rightT_sb.append(t)

            # ---- projections ---------------------------------------------
            # q^T (per head: (64, M)) and k^T (per head: (64, N)); heads are
            # produced two at a time (128 psum partitions).
            qT = [sing.tile([64, M], bf16, tag=f"qT{h}", name=f"qT{h}") for h in range(H)]
            kT = [sing.tile([64, N], bf16, tag=f"kT{h}", name=f"kT{h}") for h in range(H)]
            u_sb = [sing.tile([128, M], bf16, tag=f"u{p}", name=f"u{p}") for p in range(KT2)]
            v_aug = sing.tile([128, NT, H, DA], bf16, tag="vaug")
            nc.vector.memset(v_aug[:, :, :, DH : DH + 1], 1.0)

            # q projection (upfront; cheap and needed first)
            for t2 in range(H // 2):
                ps = spool.tile([128, SW], f32, tag="s")
                w512 = min(512, M)
                for mh in range(M // w512):
                    for kt in range(KT):
                        nc.tensor.matmul(
                            ps[:, mh * w512 : (mh + 1) * w512],
                            lhsT=wq_sb[:, kt, t2 * 128 : (t2 + 1) * 128],
                            rhs=leftT_sb[kt][:, mh * w512 : (mh + 1) * w512],
                            start=(kt == 0),
                            stop=(kt == KT - 1),
                        )
                nc.scalar.copy(out=qT[2 * t2][:, :], in_=ps[0:64, 0:M])
                nc.scalar.copy(out=qT[2 * t2 + 1][:, :], in_=ps[64:128, 0:M])

            CW = min(SW, N)
            NKC = N // CW

            def k_chunk(t2, cp):
                """k-projection for one head pair, one N-chunk: emits the
                8 matmuls + 2 evacuation copies."""
                ps = spool.tile([128, SW], f32, tag="s", name="kps")
                w512 = min(512, CW)
                for half in range(CW // w512):
                    for kt in range(KT):
                        nc.tensor.matmul(
                            ps[:, half * w512 : (half + 1) * w512],
                            lhsT=wk_sb[:, kt, t2 * 128 : (t2 + 1) * 128],
                            rhs=rightT_sb[kt][
                                :, cp * CW + half * w512 : cp * CW + (half + 1) * w512
                            ],
                            start=(kt == 0),
                            stop=(kt == KT - 1),
                        )
                nc.vector.tensor_copy(
                    out=kT[2 * t2][:, cp * CW : (cp + 1) * CW], in_=ps[0:64, 0:CW]
                )
                nc.vector.tensor_copy(
                    out=kT[2 * t2 + 1][:, cp * CW : (cp + 1) * CW],
                    in_=ps[64:128, 0:CW],
                )

            def v_nt(nt):
                """v-projection for one kv-token tile nt."""
                ps = spool.tile([128, SW], f32, tag="s", name="vps")
                for kt in range(KT):
                    nc.tensor.matmul(
                        ps[:, 0:VW],
                        lhsT=rightT_sb[kt][:, nt * 128 : (nt + 1) * 128],
                        rhs=wv_sb[:, kt, :],
                        start=(kt == 0),
                        stop=(kt == KT - 1),
                    )
                nc.vector.tensor_copy(
                    out=v_aug[:, nt, :, 0:DH], in_=ps[:, 0:VW]
                )

            def outproj_mt(mt):
                """output projection for one 128-row m-slice; reuses the
                freed O-accumulator PSUM banks."""
                ps = opool.tile([128, 512], f32, tag=f"o{mt % 2}", name="ops")
                for p2 in range(KT2):
                    nc.tensor.matmul(
                        ps[:, 0:DQ],
                        lhsT=u_sb[p2][:, mt * 128 : (mt + 1) * 128],
                        rhs=wout_sb[:, p2, :],
                        start=(p2 == 0),
                        stop=(p2 == KT2 - 1),
                    )
                ob = outp.tile([128, DQ], f32, tag="ob")
                nc.vector.tensor_copy(ob, ps[:, 0:DQ])
                nc.sync.dma_start(out=out_p[mt * 128 : (mt + 1) * 128, :], in_=ob)

            # upfront prerequisites for attention (mc0, hp0): k for heads
            # 0/1, first v tiles.  The rest interleaves into the attention
            # loop below (PE slack absorbs it).
            UPFRONT_V = min(NT, 8)
            for cp in range(NKC):
                k_chunk(0, cp)
            for nt in range(UPFRONT_V):
                v_nt(nt)
            deferred = [lambda nt=nt: v_nt(nt) for nt in range(UPFRONT_V, NT)]
            if H > 2:
                deferred += [lambda cp=cp: k_chunk(1, cp) for cp in range(NKC)]

            # ---- attention ----------------------------------------------
            # head-pair outer loop keeps PSUM pressure low (2 O accumulators
            # + 3 double-wide S tiles = 8 banks) so the S pipeline stays deep.
            DEPTH = 3
            for mc in range(NMC):
                msks = []
                for ntp in range(NTP):
                    msk = mpool.tile([128, SW], bf16, tag="msk", name=f"msk{ntp}")
                    nc.sync.dma_start(
                        out=msk,
                        in_=maskT[
                            ntp * 256 : (ntp + 1) * 256, mc * MCH : (mc + 1) * MCH
                        ].rearrange("(a p) f -> p a f", p=128),
                    )
                    msks.append(msk)
                for hp in range(H // 2):
                    o_ps = [
                        opool.tile([DA, MCH], f32, tag=f"o{i}", name=f"o{i}")
                        for i in range(2)
                    ]
                    # software pipeline: O-matmuls trail the S/exp/mask chain
                    # by DEPTH steps so the PE never stalls on exp/mask deps.
                    oq = []          # deferred (pm, i, ntp)
                    started = [False, False]

                    def flush_one(o_ps=None, oq=None, started=None, hp=None):
                        pass

                    def make_flush(o_ps, oq, started, hp):
                        def flush_one():
                            pm, i, ntp_ = oq.pop(0)
                            for half in range(2):
                                nt = 2 * ntp_ + half
                                nc.tensor.matmul(
                                    o_ps[i],
                                    lhsT=v_aug[:, nt, 2 * hp + i, :],
                                    rhs=pm[:, half * MCH : (half + 1) * MCH],
                                    start=(not started[i]),
                                    stop=(ntp_ == NTP - 1 and half == 1),
                                )
                                started[i] = True
                        return flush_one

                    flush_one = make_flush(o_ps, oq, started, hp)

                    for ntp in range(NTP):
                        for i in range(2):
                            h = 2 * hp + i
                            s_ps = spool.tile([128, SW], f32, tag="s")
                            for half in range(2):
                                nt = 2 * ntp + half
                                nc.tensor.matmul(
                                    s_ps[:, half * MCH : (half + 1) * MCH],
                                    lhsT=kT[h][:, nt * 128 : (nt + 1) * 128],
                                    rhs=qT[h][:, mc * MCH : (mc + 1) * MCH],
                                    start=True,
                                    stop=True,
                                )
                            p_sb = ppool.tile([128, SW], bf16, tag="p")
                            nc.scalar.activation(p_sb, s_ps, EXP)
                            pm = ppool.tile([128, SW], bf16, tag="pm")
                            nc.vector.tensor_mul(pm, p_sb, msks[ntp])
                            oq.append((pm, i, ntp))
                            if len(oq) > DEPTH:
                                flush_one()
                            if deferred:
                                deferred.pop(0)()
                    while oq:
                        flush_one()
                    # normalize: U^T = O^T[0:DH] * (1/d) broadcast over parts
                    for i in range(2):
                        h = 2 * hp + i
                        rdc = smallp.tile([1, MCH], f32, tag="rdc", name=f"rdc{i}")
                        nc.vector.tensor_copy(out=rdc, in_=o_ps[i][DH : DH + 1, :])
                        rd = smallp.tile([1, MCH], f32, tag="rd", name=f"rd{i}")
                        nc.vector.reciprocal_approx_fast(out=rd, in_=rdc)
                        bd = smallp.tile([64, MCH], f32, tag="bd", name=f"bd{i}")
                        nc.gpsimd.partition_broadcast(bd, rd)
                        nc.vector.tensor_mul(
                            u_sb[h // 2][
                                (h % 2) * 64 : (h % 2) * 64 + 64,
                                mc * MCH : (mc + 1) * MCH,
                            ],
                            o_ps[i][0:DH, :],
                            bd,
                        )
                # after this m-chunk is normalized, its output projection
                # becomes available work; interleave it into the next chunk
                for mt in range(mc * MCH // 128, (mc + 1) * MCH // 128):
                    deferred.append(lambda mt=mt: outproj_mt(mt))

            # flush any remaining deferred work (last m-chunk's out-proj)
            while deferred:
                deferred.pop(0)()

    nc.finalize()
    return nc


_NC_CACHE = {}


def _get_nc(key=(1024, 4096, 512, 4, 64)):
    if key not in _NC_CACHE:
        _NC_CACHE[key] = build_core(*key)
    return _NC_CACHE[key]


def kernel(left, right, mask, Wq, Wkv, Wout, bout):
    """Full-input entry point: shards across 8 neuron cores, returns the
    full (B, M, DQ) output."""
    global LAST_RESULTS
    B, M, DQmat = left.shape
    _, N, DC = right.shape
    H, DH = 8, 64
    D = H * DH
    Hc = H // 2          # heads per core
    scale = DH ** -0.5

    left = np.asarray(left, dtype=np.float32)
    right = np.asarray(right, dtype=np.float32)
    Wq = np.asarray(Wq, dtype=np.float32)
    Wkv = np.asarray(Wkv, dtype=np.float32)
    Wout = np.asarray(Wout, dtype=np.float32)
    bout = np.asarray(bout, dtype=np.float32)

    Wqs = (Wq * scale).astype(BF16)            # (DQ, D)
    Wk = Wkv[:, :D].astype(BF16)               # (DC, D)
    Wv = Wkv[:, D:].astype(BF16)               # (DC, D)
    WoutB = Wout.astype(BF16)                  # (D, DQ)

    leftT = np.ascontiguousarray(left.transpose(0, 2, 1)).astype(BF16)    # (B, DQ, M)
    rightT = np.ascontiguousarray(right.transpose(0, 2, 1)).astype(BF16)  # (B, DC, N)
    maskT = np.ascontiguousarray(mask.transpose(0, 2, 1)).astype(BF16)    # (B, N, M)

    nc = _get_nc((M, N, DQmat, Hc, DH))

    in_maps = []
    for core in range(8):
        b, hh = divmod(core, 2)
        hs = slice(hh * Hc * DH, (hh + 1) * Hc * DH)
        in_maps.append(
            {
                "leftT": leftT[b],
                "rightT": rightT[b],
                "maskT": maskT[b],
                "wq": np.ascontiguousarray(Wqs[:, hs]),
                "wk": np.ascontiguousarray(Wk[:, hs]),
                "wv": np.ascontiguousarray(Wv[:, hs]),
                "wout": np.ascontiguousarray(WoutB[hs, :]),
            }
        )

    tmpdir = None
    if TRACE:
        import shutil, tempfile

        shutil.rmtree("/tmp/attn_trace", ignore_errors=True)
        tmpdir = "/tmp/attn_trace"
    res = run_bass_kernel_spmd(nc, in_maps, list(range(8)), trace=TRACE, tmpdir=tmpdir)
    LAST_RESULTS = res

    out = np.zeros((B, M, DQmat), np.float32)
    for core in range(8):
        out[core // 2] += res.results[core]["out_p"]
    out += bout[None, None, :]
    return out
